# revision 66
# baseline (speedup 1.0000x reference)
"""GAT-style dense attention kernel for TRN2 (8 NeuronCores, SPMD over batch).

Reference computation (B=N=256, F=128, H=4, D=8):
  q = x@Wq+bq; k = x@Wk+bk; v = x@Wv+bv          (per-head dim D=8)
  s = einsum('bqhd,bkhd->bhqk', q, k)/sqrt(D)
  s = where(adj[q,k]==0, -inf, s)                 (adj shared across b,h)
  a = softmax(s, -1)
  out = einsum('bhqk,bkhd->bqhd', a, v).reshape(B,N,H*D) @ Wo + bo

Kernel strategy (per core: 32 batches):
  - host: xT = x.transpose -> [b, F, N]; contraction dim F on partitions
  - qT/kT "spread" [128, 512]: head h occupies partitions 32h..32h+10:
    rows 0-7 q/k dims (scale 1/sqrt(D) folded), row 8 = q.bk cross term
    (q side) / ones (k side), row 9 = ones (q side) / k.bq (k side).
    The ones rows are produced by a per-partition +1 in the PSUM->SBUF
    copy (tensor_scalar add); the resulting constant +2 shift of every
    score is softmax-invariant.  This folds both projection biases into
    the score matmul with zero extra matmuls.
  - scores S^T[k,q] per head: K=10 matmuls, 4 heads packed in PE rows
  - mask addend (-20 on non-edges) written into PSUM by fp8e4 DoubleRow
    identity-matmuls (0.5 cycles/col) for heads exp'd on ACT
  - exp: split across engines.  ACT heads: native Exp out of PSUM ->
    bf16.  DVE/Pool heads: Schraudolph bf16 exp (one scalar_tensor_tensor:
    int16(round(s*A16 + plane)) bitcast bf16, mask folded into plane)
  - V and Wo fused on host (Wvo per head + ones column for row sums)
  - attention application uses E as the matmul *stationary* operand:
    out[q, 9h+j] += sum_k E[k,q] * vw[k, j]  -- 16 matmuls of 9 columns,
    output directly in natural [q, :] layout (no PE transpose needed)
  - normalize: reciprocal of row sums + scale + head-sum + bias, batched
    8 batches per pass on DVE; output staged and DMA'd per 8 batches
"""

import sys

sys.path.insert(0, "/opt/trn_rl_repo")

import numpy as np

import concourse.bass as bass
import concourse.tile as tile
from concourse import mybir
from concourse.bass import ts
from concourse.bass_utils import run_bass_kernel_spmd
from concourse.tile_rust import add_dep_helper


def _dep(from_inst, to_inst, reason):
    if from_inst is None or to_inst is None:
        return
    add_dep_helper(
        getattr(from_inst, "ins", from_inst),
        getattr(to_inst, "ins", to_inst),
        sync=False,
        reason=reason,
    )

B = 256
N = 256
F = 128
H = 4
D = 8
NCORES = 8
BPC = B // NCORES  # batches per core
MASK_NEG = -20.0

# Schraudolph bf16-exp constants: y_i16 = round(s*A16 + B16 (+ A16*mask)),
# bitcast int16 -> bf16 approximates exp(s + mask).
A16 = 184.6618  # 2^7 / ln 2
B16 = 16250.5

f32 = mybir.dt.float32
f32r = mybir.dt.float32r
bf16 = mybir.dt.bfloat16
i16 = mybir.dt.int16
fp8 = mybir.dt.float8e4

# exp assignment: heads 0+1 share one 2-bank PSUM tile and a single ACT
# exp instruction (fp8 DoubleRow additive mask in PSUM); head 3 is ACT
# with its own tile; head 2 uses the DVE Schraudolph bf16 exp with the
# mask folded into the additive plane.  GPSIMD cannot access PSUM on
# real hardware, so Pool only gets SBUF work (the per-8-batch normalize).


def _pack_bf16_as_f32(a):
    """[P, 2n] bf16 values -> [P, n] f32 bit-pattern columns."""
    import ml_dtypes

    ab = a.astype(ml_dtypes.bfloat16).view(np.uint16).astype(np.uint32)
    assert ab.shape[1] % 2 == 0
    packed = ab[:, 0::2] | (ab[:, 1::2] << 16)
    return packed.view(np.float32)


def _pack_fp8_as_f32(a):
    """[P, 4n] fp8e4m3 values -> [P, n] f32 bit-pattern columns."""
    import ml_dtypes

    ab = a.astype(ml_dtypes.float8_e4m3fn).view(np.uint8).astype(np.uint32)
    assert ab.shape[1] % 4 == 0
    packed = (
        ab[:, 0::4]
        | (ab[:, 1::4] << 8)
        | (ab[:, 2::4] << 16)
        | (ab[:, 3::4] << 24)
    )
    return packed.view(np.float32)


def _build_consts(edge_index, Wq, bq, Wk, bk, Wv, bv, Wo, bo):
    scale = 1.0 / np.sqrt(np.float32(D))

    # spread projection weights [F, 128]: out partition 32h+r
    #   r<8: q/k dim r;  r=8: q-side q.bk cross col / k-side zero;
    #   r=9: k-side k.bq cross col / q-side zero
    Wq_s = np.zeros((F, 128), np.float32)
    Wk_s = np.zeros((F, 128), np.float32)
    for h in range(H):
        wq_h = Wq[:, 8 * h : 8 * h + 8]
        wk_h = Wk[:, 8 * h : 8 * h + 8]
        for d in range(D):
            Wq_s[:, 32 * h + d] = wq_h[:, d] * scale
            Wk_s[:, 32 * h + d] = wk_h[:, d]
        Wq_s[:, 32 * h + 8] = (wq_h @ bk[8 * h : 8 * h + 8]) * scale
        Wk_s[:, 32 * h + 9] = (wk_h @ bq[8 * h : 8 * h + 8]) * scale

    # one-hot +1 rows: q side ones at r=9 needs +1 there; k side at r=8.
    # tensor_scalar adds per-partition across both halves; +1 on rows 8,9
    # yields a constant +2 score shift (softmax invariant).
    onehot = np.zeros((128, 1), np.float32)
    for h in range(H):
        onehot[32 * h + 8, 0] = 1.0
        onehot[32 * h + 9, 0] = 1.0


    # adjacency mask, [k, q] orientation, chunked on k
    adj = np.zeros((B, B), np.float32)
    adj[edge_index[0], edge_index[1]] = 1.0
    maskT = adj.T  # [k, q]
    maskT_c = maskT.reshape(2, 128, 256).transpose(1, 0, 2)  # [128, c, q]

    # fp8 DoubleRow mask addend planes [128, (c, ktile, q)]; ktile1 zeros
    maskdr = np.zeros((128, 2, 2, 256), np.float32)
    maskdr[:, :, 0, :] = np.where(maskT_c == 0.0, np.float32(MASK_NEG), 0.0)
    # fp8 DoubleRow identity [128, (ktile, col)]; ktile1 zeros
    identdr = np.zeros((128, 2, 128), np.float32)
    identdr[:, 0, :] = np.eye(128, dtype=np.float32)

    # Schraudolph plane [128, (c, q)] f32
    msch = np.where(
        maskT_c == 0.0, np.float32(B16 + A16 * MASK_NEG), np.float32(B16)
    )

    bob = np.broadcast_to(bo.astype(np.float32), (128, D)).copy()

    cols = []
    index = {}

    def add(name, arr):
        arr = np.ascontiguousarray(arr.astype(np.float32))
        index[name] = (sum(c.shape[1] for c in cols), arr.shape[1])
        cols.append(arr)

    add("msch", msch.reshape(128, 512))             # 512 cols
    add("bob", bob)                                  # 8 cols
    add("onehot", onehot)                            # 1 col
    cblob = np.concatenate(cols, axis=1)
    import ml_dtypes

    wqk16 = np.ascontiguousarray(
        np.concatenate([Wq_s, Wk_s], axis=1).astype(ml_dtypes.bfloat16)
    )  # [128, 256] bf16
    mask8 = np.ascontiguousarray(
        maskdr.reshape(128, 1024).astype(ml_dtypes.float8_e4m3fn)
    )
    ident8 = np.ascontiguousarray(
        identdr.reshape(128, 256).astype(ml_dtypes.float8_e4m3fn)
    )
    return {
        "cblob": np.ascontiguousarray(cblob),
        "wqk16": wqk16,
        "mask8": mask8,
        "ident8": ident8,
    }, index


CIDX = None  # filled by _build_consts; layout is deterministic


def _split_excess_waits(nc, max_waits=1):
    """Walrus allows only 2 sync-wait slots per engine instruction. Tile's
    vector-clock wait emission occasionally exceeds that (schedule-dependent);
    hoist the excess onto injected same-engine NoOps placed just before."""
    f = nc.m.functions[0]
    for bb in f.blocks:
        insts = list(bb.instructions)
        n_inserted = 0
        for idx, inst in enumerate(insts):
            si = getattr(inst, "sync_info", None)
            if si is None or not si.on_wait or len(si.on_wait) <= max_waits:
                continue
            waits = list(si.on_wait)
            keep, excess = waits[:max_waits], waits[max_waits:]
            pos = idx + n_inserted
            while excess:
                chunk, excess = excess[:max_waits], excess[max_waits:]
                nop = mybir.InstNoOp(
                    name=nc.get_next_instruction_name(),
                    ins=[],
                    outs=[],
                    engine=inst.engine,
                    sync_info=mybir.SyncInfo(on_wait=chunk, on_update=[]),
                    bass_nofuse=True,
                )
                bb.instructions.insert(pos, nop)
                pos += 1
                n_inserted += 1
            inst.sync_info = mybir.SyncInfo(on_wait=keep, on_update=si.on_update)


def _build_program(cidx):
    nc = bass.Bass()

    ncols = cidx["onehot"][0] + cidx["onehot"][1]
    import os
    dbg = os.environ.get("KDBG", "0") == "1"
    x_t = nc.declare_dram_parameter("xt", [BPC, F, N], bf16, isOutput=False)
    if dbg:
        dbg_qk = nc.declare_dram_parameter("dbg_qk", [128, 512], f32, isOutput=True)
        dbg_e = nc.declare_dram_parameter("dbg_e", [H, 128, 512], f32, isOutput=True)
        dbg_pst = nc.declare_dram_parameter("dbg_pst", [128, 2, 36], f32, isOutput=True)
        dbg_rec = nc.declare_dram_parameter("dbg_rec", [128, 2, H], f32, isOutput=True)
    vw_t = nc.declare_dram_parameter("vwt", [BPC, 128, 2, 9 * H], bf16, isOutput=False)
    out = nc.declare_dram_parameter("out", [BPC, N, D], f32, isOutput=True)
    c_blob = nc.declare_dram_parameter("cblob", [128, ncols], f32r, isOutput=False)
    c_wqk = nc.declare_dram_parameter("wqk16", [128, 256], bf16, isOutput=False)
    c_mask8 = nc.declare_dram_parameter("mask8", [128, 1024], fp8, isOutput=False)
    c_ident8 = nc.declare_dram_parameter("ident8", [128, 256], fp8, isOutput=False)

    def creg(name, dtype=None, shape=None):
        off, width = cidx[name]
        ap = cblob[:, off : off + width]
        if dtype is not None:
            ap = ap.bitcast(dtype)
        if shape is not None:
            ap = ap.rearrange(shape[0], **shape[1])
        return ap

    with tile.TileContext(nc) as tc:
        with (
            tc.tile_pool(name="consts", bufs=1) as cpool,
            tc.tile_pool(name="xt", bufs=16) as xt_pool,
            tc.tile_pool(name="qk", bufs=4) as qk_pool,
            tc.tile_pool(name="vw", bufs=4) as vw_pool,
            tc.tile_pool(name="E", bufs=20) as e_pool,
            tc.tile_pool(name="small", bufs=4) as sm_pool,
            tc.tile_pool(name="stage", bufs=2) as st_pool,
            tc.tile_pool(name="ps_qk", bufs=1, space="PSUM") as ps_qk_pool,
            tc.tile_pool(name="ps_s", bufs=5, space="PSUM") as ps_s_pool,
            tc.tile_pool(name="ps_p9", bufs=2, space="PSUM") as ps_p9_pool,
        ):
            wqk_sb = cpool.tile([128, 256], bf16, tag="wqk")
            nc.sync.dma_start(out=wqk_sb[:], in_=c_wqk[:])
            cblob = cpool.tile([128, ncols], f32r, tag="cblob")
            ident_sb = cpool.tile([128, 256], fp8, tag="ident8")
            mask_sb = cpool.tile([128, 1024], fp8, tag="mask8")

            wqs = wqk_sb[:, 0:128]
            wks = wqk_sb[:, 128:256]
            maskdr = mask_sb[:].rearrange("p (c t q) -> p c t q", c=2, t=2)
            identdr = ident_sb[:].rearrange("p (t c) -> p t c", t=2)
            msch = creg("msch")                # [128, 512] f32 bits (c, q)
            msch_f = msch.bitcast(f32)
            bob = creg("bob", f32)             # [128, 8]
            onehot = creg("onehot", f32)       # [128, 1]

            # Make DVE/ACT/Pool observe the const-DMA queue once so the
            # const-load tick drops out of later wait lists.
            obs = cpool.tile([1, 8], f32, tag="obs")
            nc.vector.tensor_copy(obs[:, 0:2], wqk_sb[0:1, 0:4].bitcast(f32))
            nc.scalar.copy(obs[:, 2:4], wqk_sb[0:1, 4:8].bitcast(f32))
            nc.gpsimd.tensor_copy(obs[:, 4:6], wqk_sb[0:1, 8:12].bitcast(f32))

            recst = None
            pst = None
            ostage = None
            xt_tiles = {}
            e_tiles = {}
            st = {}

            def load_pair(b0):
                t = xt_pool.tile([128, 2, 2, 128], bf16, tag="xt")
                nc.sync.dma_start(
                    out=t[:],
                    in_=x_t[b0 : b0 + 2].rearrange("b f (c n) -> f b c n", c=2),
                )
                xt_tiles[b0] = t

            vw_tiles = {}

            def load_vw(b0):
                t = vw_pool.tile([128, 8, 2, 9 * H], bf16, tag="vw")
                nc.sync.dma_start(
                    out=t[:],
                    in_=vw_t[b0 : b0 + 8].rearrange("b p c j -> p b c j"),
                )
                vw_tiles[b0] = t

            def stage_a(b):
                # projections + PSUM->SBUF copies for batch b
                xt_b = xt_tiles[(b // 2) * 2][:, b % 2]  # [128, 2, 128]
                xt_flat = xt_b.rearrange("p c n -> p (c n)")
                ps_qk = ps_qk_pool.tile([128, 512], f32, tag="qk")
                nc.tensor.matmul(
                    ps_qk[:, 0:256], wqs, xt_flat, start=True, stop=True,
                )
                nc.tensor.matmul(
                    ps_qk[:, 256:512], wks, xt_flat,
                    start=True, stop=True, skip_group_check=True,
                )
                qk_sb = qk_pool.tile([128, 512], bf16, tag="qk")
                nc.vector.tensor_scalar_add(qk_sb[:], ps_qk[:], onehot)
                if dbg and b == 0:
                    qkf = qk_pool.tile([128, 512], f32, tag="qkdbg")
                    nc.vector.tensor_copy(qkf[:], qk_sb[:])
                    nc.sync.dma_start(out=dbg_qk[:], in_=qkf[:])
                st[("qk", b)] = qk_sb

            def stage_b(b):
                # scores + exp for batch b
                qk_sb = st[("qk", b)]
                msch_cq = msch_f[:, 0:512].rearrange("p (c q) -> p c q", c=2)
                tail = False

                # head 3: c0 ACT native (masked), c1 DVE Schraudolph
                r0 = 96
                ps_h3 = ps_s_pool.tile([128, 2, 256], f32, tag="S")
                for c in range(2):
                    if c == 0 or tail:
                        nc.tensor.matmul(
                            ps_h3[:, c, :],
                            identdr, maskdr[:, c],
                            start=True, stop=False,
                            perf_mode=mybir.MatmulPerfMode.DoubleRow,
                            skip_group_check=True,
                        )
                    nc.tensor.matmul(
                        ps_h3[:, c, :],
                        qk_sb[r0 : r0 + 10, 256 + 128 * c : 384 + 128 * c],
                        qk_sb[r0 : r0 + 10, 0:256],
                        start=(c == 1 and not tail), stop=True,
                        skip_group_check=True,
                        tile_position=(r0, 0),
                    )
                e_h3 = e_pool.tile([128, 2, 256], bf16, tag="E")
                if tail:
                    nc.scalar.activation(
                        e_h3[:], ps_h3[:],
                        mybir.ActivationFunctionType.Exp,
                    )
                else:
                    nc.scalar.activation(
                        e_h3[:, 0, :], ps_h3[:, 0, :],
                        mybir.ActivationFunctionType.Exp,
                    )
                    nc.vector.scalar_tensor_tensor(
                        e_h3[:, 1, :].bitcast(i16),
                        ps_h3[:, 1, :],
                        float(A16),
                        msch_f[:, 256:512],
                        op0=mybir.AluOpType.mult,
                        op1=mybir.AluOpType.add,
                    )
                e_tiles[(b, 3)] = e_h3

                # head 2: DVE Schraudolph (mask in additive plane); the
                # tail batches go to ACT (native exp) since ACT drains first
                r0 = 64
                ps_h2 = ps_s_pool.tile([128, 2, 256], f32, tag="S")
                for c in range(2):
                    if tail:
                        nc.tensor.matmul(
                            ps_h2[:, c, :],
                            identdr, maskdr[:, c],
                            start=True, stop=False,
                            perf_mode=mybir.MatmulPerfMode.DoubleRow,
                            skip_group_check=True,
                        )
                    nc.tensor.matmul(
                        ps_h2[:, c, :],
                        qk_sb[r0 : r0 + 10, 256 + 128 * c : 384 + 128 * c],
                        qk_sb[r0 : r0 + 10, 0:256],
                        start=not tail, stop=True,
                        skip_group_check=True,
                        tile_position=(r0, 0),
                    )
                e_h2 = e_pool.tile([128, 2, 256], bf16, tag="E")
                if tail:
                    nc.scalar.activation(
                        e_h2[:], ps_h2[:],
                        mybir.ActivationFunctionType.Exp,
                    )
                else:
                    nc.vector.scalar_tensor_tensor(
                        e_h2[:].bitcast(i16),
                        ps_h2[:],
                        float(A16),
                        msch_cq,
                        op0=mybir.AluOpType.mult,
                        op1=mybir.AluOpType.add,
                    )
                e_tiles[(b, 2)] = e_h2

                # heads 0, 1: ACT native exp, masked, own bank each
                for h01 in (0, 1):
                    r0 = 32 * h01
                    ps_h = ps_s_pool.tile([128, 2, 256], f32, tag="S")
                    for c in range(2):
                        nc.tensor.matmul(
                            ps_h[:, c, :],
                            identdr, maskdr[:, c],
                            start=True, stop=False,
                            perf_mode=mybir.MatmulPerfMode.DoubleRow,
                            skip_group_check=True,
                        )
                        nc.tensor.matmul(
                            ps_h[:, c, :],
                            qk_sb[r0 : r0 + 10, 256 + 128 * c : 384 + 128 * c],
                            qk_sb[r0 : r0 + 10, 0:256],
                            start=False, stop=True,
                            skip_group_check=True,
                            tile_position=(r0, 0),
                        )
                    e_h01 = e_pool.tile([128, 2, 256], bf16, tag="E")
                    nc.scalar.activation(
                        e_h01[:], ps_h[:],
                        mybir.ActivationFunctionType.Exp,
                    )
                    e_tiles[(b, h01)] = e_h01

            def stage_c(b):
                # attention apply + normalize prep for batch b
                nonlocal recst, pst, ostage
                vw_sb = vw_tiles[(b // 8) * 8]
                if b % 4 == 0:
                    p9_four = ps_p9_pool.tile([128, 4, 72], f32, tag="p9")
                    st[("p9",)] = p9_four
                ps_p9 = st[("p9",)][:, b % 4]
                for h in range(H):
                    e_h = e_tiles.pop((b, h))
                    for qc in range(2):
                        for c in range(2):
                            nc.tensor.matmul(
                                ps_p9[:, 36 * qc + 9 * h : 36 * qc + 9 * h + 9],
                                e_h[:, c, 128 * qc : 128 * qc + 128],
                                vw_sb[:, b % 8, c, 9 * h : 9 * h + 9],
                                start=(c == 0), stop=(c == 1),
                                skip_group_check=True,
                            )
                if b % 8 == 0:
                    recst = sm_pool.tile([128, 8, 2, H], f32, tag="rec")
                    pst = st_pool.tile([128, 8, 2, 36], f32, tag="pst")
                    ostage = st_pool.tile([128, 8, 2, D], f32, tag="ost")
                if b % 4 == 3:
                    p9f = st[("p9",)].rearrange("p b (qc v) -> p b qc v", qc=2)
                    nc.scalar.copy(pst[:, b % 8 - 3 : b % 8 + 1], p9f[:])
                    psth = pst[:, b % 8 - 3 : b % 8 + 1].rearrange(
                        "p b qc (h n) -> p b qc h n", h=4
                    )
                    nc.vector.reciprocal(
                        recst[:, b % 8 - 3 : b % 8 + 1], psth[:, :, :, :, 0]
                    )
                if dbg and b == 0:
                    nc.sync.dma_start(out=dbg_pst[:], in_=pst[:, 0])
                    nc.sync.dma_start(out=dbg_rec[:], in_=recst[:, 0])
                if b % 4 == 3:
                    half = (b % 8) // 4
                    s0, s1 = 4 * half, 4 * half + 4
                    eng = nc.vector if b == BPC - 1 else nc.gpsimd
                    tmp = sm_pool.tile([128, 4, 2, D, H], f32, tag="tmp")
                    eng.tensor_tensor(
                        tmp[:],
                        pst[:, s0:s1].rearrange(
                            "p b c (h j) -> p b c j h", h=H
                        )[:, :, :, 1:9, :],
                        recst[:, s0:s1].unsqueeze(3).to_broadcast(
                            [128, 4, 2, D, H]
                        ),
                        op=mybir.AluOpType.mult,
                    )
                    t01 = sm_pool.tile([128, 4, 2, D], f32, tag="t01")
                    if eng is nc.vector:
                        # last group: head-sum via one free-axis reduce (DVE
                        # only; gpsimd cannot reduce along X)
                        nc.vector.tensor_reduce(
                            t01[:], tmp[:], axis=mybir.AxisListType.X,
                            op=mybir.AluOpType.add,
                        )
                    else:
                        t23 = sm_pool.tile([128, 4, 2, D], f32, tag="t23")
                        eng.tensor_tensor(
                            t01[:], tmp[:, :, :, :, 0], tmp[:, :, :, :, 1],
                            op=mybir.AluOpType.add,
                        )
                        eng.tensor_tensor(
                            t23[:], tmp[:, :, :, :, 2], tmp[:, :, :, :, 3],
                            op=mybir.AluOpType.add,
                        )
                        eng.tensor_tensor(
                            t01[:], t01[:], t23[:], op=mybir.AluOpType.add,
                        )
                    eng.tensor_tensor(
                        ostage[:, s0:s1],
                        t01[:],
                        bob.unsqueeze(1).unsqueeze(1).to_broadcast(
                            [128, 4, 2, D]
                        ),
                        op=mybir.AluOpType.add,
                    )
                if b % 4 == 3:
                    nc.sync.dma_start(
                        out=out[b - 3 : b + 1].rearrange(
                            "b (c p) j -> p b c j", c=2
                        ),
                        in_=ostage[:, 4 * half : 4 * half + 4],
                    )

            # software-pipelined schedule: A(i) | B(i-1) | C(i-2)
            load_pair(0)
            nc.sync.dma_start(out=cblob[:], in_=c_blob[:])
            nc.sync.dma_start(out=ident_sb[:], in_=c_ident8[:])
            nc.sync.dma_start(out=mask_sb[:], in_=c_mask8[:])
            load_vw(0)
            for b0 in range(2, BPC, 2):
                load_pair(b0)
                if b0 % 8 == 2 and b0 + 6 < BPC:
                    load_vw(b0 + 6)
            for i in range(BPC + 2):
                if i < BPC:
                    stage_a(i)
                if 0 <= i - 1 < BPC:
                    stage_b(i - 1)
                if 0 <= i - 2 < BPC:
                    stage_c(i - 2)

    _split_excess_waits(nc)
    return nc


_NC_CACHE = None
LAST_RESULTS = None


def kernel(**inputs) -> np.ndarray:
    global _NC_CACHE, CIDX
    x = np.asarray(inputs["x"], np.float32)
    edge_index = np.asarray(inputs["edge_index"])
    consts, cidx = _build_consts(
        edge_index,
        np.asarray(inputs["Wq"], np.float32), np.asarray(inputs["bq"], np.float32),
        np.asarray(inputs["Wk"], np.float32), np.asarray(inputs["bk"], np.float32),
        np.asarray(inputs["Wv"], np.float32), np.asarray(inputs["bv"], np.float32),
        np.asarray(inputs["Wo"], np.float32), np.asarray(inputs["bo"], np.float32),
    )
    CIDX = cidx

    if _NC_CACHE is None:
        _NC_CACHE = _build_program(cidx)
    nc = _NC_CACHE

    # host-side fused V*Wo projection (+ bias, + ones column for row sums)
    import ml_dtypes

    Wv = np.asarray(inputs["Wv"], np.float32)
    Wo = np.asarray(inputs["Wo"], np.float32)
    bv = np.asarray(inputs["bv"], np.float32)
    Wvo = np.zeros((F, 9 * H), np.float32)
    bvo = np.zeros(9 * H, np.float32)
    for h in range(H):
        wv_h = Wv[:, 8 * h : 8 * h + 8]
        wo_h = Wo[8 * h : 8 * h + 8, :]
        Wvo[:, 9 * h + 1 : 9 * h + 9] = wv_h @ wo_h
        bvo[9 * h + 1 : 9 * h + 9] = bv[8 * h : 8 * h + 8] @ wo_h
        bvo[9 * h] = 1.0
    vw_full = x.reshape(B * N, F) @ Wvo + bvo  # [B*N, 36]
    vw_full = vw_full.reshape(B, N, 9 * H).astype(ml_dtypes.bfloat16)

    in_maps = []
    for core in range(NCORES):
        xs = x[core * BPC : (core + 1) * BPC]  # [BPC, N, F]
        xt = np.ascontiguousarray(
            xs.transpose(0, 2, 1).astype(ml_dtypes.bfloat16)
        )  # [BPC, F, N]
        # vwt[b, p, c, j] = vw[b, k = c*128 + p, j]
        vws = vw_full[core * BPC : (core + 1) * BPC]  # [BPC, 256, 36]
        vwt = np.ascontiguousarray(
            vws.reshape(BPC, 2, 128, 9 * H).transpose(0, 2, 1, 3)
        )
        m = {"xt": xt, "vwt": vwt}
        m.update(consts)
        in_maps.append(m)

    res = run_bass_kernel_spmd(nc, in_maps, list(range(NCORES)))
    global LAST_RESULTS
    LAST_RESULTS = res
    outs = [res.results[i]["out"] for i in range(NCORES)]
    return np.concatenate(outs, axis=0).astype(np.float32)


if __name__ == "__main__":
    rng = np.random.default_rng(0)
    demo = dict(
        x=rng.standard_normal((B, N, F), dtype=np.float32),
        edge_index=np.concatenate(
            [rng.integers(0, B, (2, 8192)), np.stack([np.arange(B)] * 2)], axis=1
        ).astype(np.int32),
        Wq=rng.standard_normal((F, H * D), dtype=np.float32) / np.sqrt(F),
        bq=rng.standard_normal(H * D, dtype=np.float32) / np.sqrt(F),
        Wk=rng.standard_normal((F, H * D), dtype=np.float32) / np.sqrt(F),
        bk=rng.standard_normal(H * D, dtype=np.float32) / np.sqrt(F),
        Wv=rng.standard_normal((F, H * D), dtype=np.float32) / np.sqrt(F),
        bv=rng.standard_normal(H * D, dtype=np.float32) / np.sqrt(F),
        Wo=rng.standard_normal((H * D, D), dtype=np.float32) / np.sqrt(H * D),
        bo=rng.standard_normal(D, dtype=np.float32) / np.sqrt(H * D),
    )
    out = kernel(**demo)
    print("kernel output", out.shape, out.dtype)
